# revision 1
# baseline (speedup 1.0000x reference)
"""Time-varying FIR (AllZeroDigitalFilter) on 8 TRN2 NeuronCores.

fp16 "C-decomposition", dual-engine (Vector + Scalar):
  C_k[i'] = sum_j h_pad[k,j] * x[(k-1)P + i' - j],  i' in [0,160)
  (filter of frame k applied across frames k-1 and k)
  y[kP+i] = w0[i]*C_k[80+i] + w1[i]*C_{k+1}[i]
This halves op count vs the direct A/B blend: one FD=160 op per tap
instead of two FD=80 ops. Per 126-row tile, N_DVE taps run as fused
mult-add chains on the Vector engine (scalar_tensor_tensor); the other
D-N_DVE tap-products run concurrently on the Scalar engine
(activation Copy with per-partition scale) into a contiguous 32-slot
product buffer, folded into the accumulator on Vector by a narrow
pre-fold (slots 16..N_ACT onto the front) plus an in-place 16-slot
halving tree — all wide fp16 2x-mode adds, ~2.3us per tile. The cross-partition (+1) combine uses a
partition-shifted SBUF->SBUF DMA + one tensor add that emits fp32
directly (DVE converts on output), so stores write the final output
with no staging or cast pass.
Precision (validated vs reference): ~7.8e-4 relative error.
Sharding: pure data parallel across batch, 2 sequences per core.

Sync design note: cumulative thresholds on a shared DMA semaphore are
unsound with >1 DMA in flight (per-SDMA-engine completion skew lets a
later tile's increments satisfy an earlier tile's threshold). Buffer-
parity semaphores make every threshold equal to the maximum possible
increment count at wait time, so a fired wait implies full completion.
"""

import sys

for p in ("/opt/trn_rl_repo", "/root/.axon_site/_ro/trn_rl_repo"):
    if p not in sys.path:
        sys.path.append(p)

import numpy as np
import concourse.bass as bass
import concourse.mybir as mybir
from concourse.ap import AP
from concourse.bass_utils import run_bass_kernel_spmd

B, T = 16, 80000
P, D = 80, 50  # frame period, taps
N = T // P  # 1000 frames
W2 = 2 * P + D - 1  # 209: extended window for the 160-wide C rows
NCORES = 8
S = B // NCORES  # sequences per core
FO = 125  # output frames per tile
FT = FO + 1  # C-rows per tile (tiles overlap by 1 row)
NTSEQ = N // FO  # 8 tiles per sequence
PAD = D - 1 + P  # front pad so C_k window starts at k*P: 129
TPC = N * P + W2 + 2  # padded x length (+2 slack for the odd-offset copy)

F16 = mybir.dt.float16
FP32 = mybir.dt.float32

N_DVE = 26  # taps computed on the Vector engine (fused mult-add chain)
# remaining D - N_DVE taps: products on the Scalar (ACT) engine, folded in
# with fp16 2x-mode tensor adds on DVE

_nc_cache = {}


def build_nc():
    if "nc" in _nc_cache:
        return _nc_cache["nc"]
    nc = bass.Bass()
    xp_ext = nc.declare_dram_parameter("xp", [S, TPC], F16, isOutput=False)
    hc_ext = nc.declare_dram_parameter("hc", [S, N + 1, D], FP32, isOutput=False)
    rr_ext = nc.declare_dram_parameter("rr", [128, 2 * P], F16, isOutput=False)
    out_ext = nc.declare_dram_parameter("out", [S, T], FP32, isOutput=True)

    NTILES = S * NTSEQ  # tile t -> seq s = t // NTSEQ, chunk ci = t % NTSEQ

    from contextlib import ExitStack

    with ExitStack() as _ctx:
        ec = _ctx.enter_context
        xa0 = ec(nc.sbuf_tensor([FT, W2], F16))
        xa1 = ec(nc.sbuf_tensor([FT, W2], F16))
        xb0 = ec(nc.sbuf_tensor([FT, W2], F16))
        xb1 = ec(nc.sbuf_tensor([FT, W2], F16))
        hh0 = ec(nc.sbuf_tensor([FT, D], FP32))
        hh1 = ec(nc.sbuf_tensor([FT, D], FP32))
        acc0 = ec(nc.sbuf_tensor([FT, 2 * P], F16))
        acc1 = ec(nc.sbuf_tensor([FT, 2 * P], F16))
        vt = ec(nc.sbuf_tensor([FT, 2 * P], F16))
        vs = ec(nc.sbuf_tensor([FO, P], F16))
        y0 = ec(nc.sbuf_tensor([FO, P], FP32))
        y1 = ec(nc.sbuf_tensor([FO, P], FP32))
        rrt = ec(nc.sbuf_tensor([128, 2 * P], F16))
        ramp_sem = ec(nc.semaphore("ramp_sem"))
        dma_e = ec(nc.semaphore("dma_e"))
        dma_o = ec(nc.semaphore("dma_o"))
        v_sem = ec(nc.semaphore("v_sem"))
        vs_sem = ec(nc.semaphore("vs_sem"))
        ya_sem = ec(nc.semaphore("ya_sem"))
        out_e = ec(nc.semaphore("out_e"))
        out_o = ec(nc.semaphore("out_o"))
        act_sem = ec(nc.semaphore("act_sem"))
        N_ACT = D - N_DVE
        NSLOT = 32  # padded to a power of two for the in-place halving tree
        assert N_ACT <= NSLOT
        prb0 = ec(nc.sbuf_tensor([FT, NSLOT * 2 * P], F16))
        prb1 = ec(nc.sbuf_tensor([FT, NSLOT * 2 * P], F16))
        prb = [prb0, prb1]
        block = ec(nc.Block())
        xa = [xa0, xa1]
        xb = [xb0, xb1]
        hh = [hh0, hh1]
        yt = [y0, y1]
        dma_s = [dma_e, dma_o]
        out_s = [out_e, out_o]

        def ydst(t):
            s, ci = t // NTSEQ, t % NTSEQ
            return AP(
                tensor=out_ext[:].tensor,
                offset=s * T + ci * FO * P,
                ap=[[P, FO], [1, P]],
            )

        @block.sync
        def _(sync):
            for t in range(NTILES):
                s, ci = t // NTSEQ, t % NTSEQ
                b = t % 2
                k0 = ci * FO
                if t >= 2:
                    sync.wait_ge(v_sem, t - 1)  # WAR: tile t-2 read its inputs
                src_a = AP(
                    tensor=xp_ext[:].tensor,
                    offset=s * TPC + k0 * P,
                    ap=[[P, FT], [1, W2]],
                )
                src_b = AP(
                    tensor=xp_ext[:].tensor,
                    offset=s * TPC + k0 * P + 1,
                    ap=[[P, FT], [1, W2]],
                )
                sync.dma_start(xa[b][:], src_a).then_inc(dma_s[b], 16)
                sync.dma_start(xb[b][:], src_b).then_inc(dma_s[b], 16)
                sync.dma_start(hh[b][:], hc_ext[s, k0 : k0 + FT, :]).then_inc(
                    dma_s[b], 16
                )
                if t == 0:
                    sync.dma_start(rrt[:], rr_ext[:]).then_inc(ramp_sem, 16)
                if t >= 1:
                    # partition-shift copy of V rows 1..FT for tile t-1
                    sync.wait_ge(v_sem, t)
                    sync.dma_start(vs[:], vt[1:FT, 0:P]).then_inc(vs_sem, 16)
                if t >= 2:
                    # store y of tile t-2 (ya available early; avoids blocking
                    # the next tile's input DMAs behind tile t-1's compute)
                    sync.wait_ge(ya_sem, t - 1)
                    sync.dma_start(ydst(t - 2), yt[(t - 2) % 2][:]).then_inc(
                        out_s[(t - 2) % 2], 16
                    )
            # tail: last tile's shift + remaining stores
            tl = NTILES - 1
            sync.wait_ge(v_sem, NTILES)
            sync.dma_start(vs[:], vt[1:FT, 0:P]).then_inc(vs_sem, 16)
            sync.wait_ge(ya_sem, NTILES - 1)
            sync.dma_start(ydst(tl - 1), yt[(tl - 1) % 2][:]).then_inc(
                out_s[(tl - 1) % 2], 16
            )
            sync.wait_ge(ya_sem, NTILES)
            sync.dma_start(ydst(tl), yt[tl % 2][:]).then_inc(out_s[tl % 2], 16)
            sync.wait_ge(out_s[tl % 2], 16 * (tl // 2 + 1))
            sync.wait_ge(out_s[1 - tl % 2], 16 * ((tl - 1) // 2 + 1))


        def src_for(buf_pair, b, j):
            # slice of the extended window for tap j, 4B-aligned via the
            # one-element-shifted copy when the natural offset is odd
            off = D - 1 - j
            if off % 2 == 0:
                return buf_pair[0][b][:, off : off + 2 * P]
            return buf_pair[1][b][:, off - 1 : off - 1 + 2 * P]

        @block.vector
        def _(vector):
            def conv(t):
                b = t % 2
                accs = [acc0, acc1]
                vector.wait_ge(dma_s[b], 48 * (t // 2 + 1))
                vector.tensor_scalar_mul(acc0[:], src_for((xa, xb), b, 0), hh[b][:, 0:1])
                cur = 0
                for j in range(1, N_DVE):
                    nxt = 1 - cur
                    vector.scalar_tensor_tensor(
                        out=accs[nxt][:],
                        in0=src_for((xa, xb), b, j),
                        scalar=hh[b][:, j : j + 1],
                        in1=accs[cur][:],
                        op0=mybir.AluOpType.mult,
                        op1=mybir.AluOpType.add,
                    )
                    cur = nxt
                # fold in the ACT-engine products: narrow pre-level folds the
                # slots beyond 16 onto the front (no zero padding needed), then
                # an in-place halving tree over the remaining 16 slots
                vector.wait_ge(act_sem, t + 1)
                if N_ACT > 16:
                    extra = N_ACT - 16
                    vector.tensor_tensor(
                        out=prb[b][:, 0 : extra * 2 * P],
                        in0=prb[b][:, 0 : extra * 2 * P],
                        in1=prb[b][:, 16 * 2 * P : N_ACT * 2 * P],
                        op=mybir.AluOpType.add,
                    )
                    width = 16 * 2 * P
                else:
                    width = NSLOT * 2 * P
                while width > 2 * P:
                    half = width // 2
                    vector.tensor_tensor(
                        out=prb[b][:, 0:half],
                        in0=prb[b][:, 0:half],
                        in1=prb[b][:, half:width],
                        op=mybir.AluOpType.add,
                    )
                    width = half
                nxt = 1 - cur
                vector.tensor_tensor(
                    out=accs[nxt][:],
                    in0=accs[cur][:],
                    in1=prb[b][:, 0 : 2 * P],
                    op=mybir.AluOpType.add,
                )
                cur = nxt
                return accs[cur]

            if N_ACT <= 16:
                # one-time zeroing of padding slots for the pure halving tree
                for pp in range(2):
                    vector.memset(prb[pp][:, N_ACT * 2 * P : NSLOT * 2 * P], 0.0)
            for t in range(NTILES):
                fin = conv(t)
                if t == 0:
                    vector.wait_ge(ramp_sem, 16)
                if t >= 1:
                    # combine tile t-1: y = V[0:FO, 80:160] + Vs
                    vector.wait_ge(vs_sem, 16 * t)
                    if t - 1 >= 2:
                        vector.wait_ge(out_s[(t - 1) % 2], 16 * ((t - 1) // 2))
                    vector.tensor_tensor(
                        out=yt[(t - 1) % 2][:],
                        in0=vt[0:FO, P : 2 * P],
                        in1=vs[:],
                        op=mybir.AluOpType.add,
                    ).then_inc(ya_sem, 1)
                # V_t = C_t * rr
                vector.tensor_tensor(
                    out=vt[:], in0=fin[:], in1=rrt[0:FT, :], op=mybir.AluOpType.mult
                ).then_inc(v_sem, 1)
            # tail combine for last tile
            tl = NTILES - 1
            vector.wait_ge(vs_sem, 16 * NTILES)
            vector.wait_ge(out_s[tl % 2], 16 * (tl // 2))
            vector.tensor_tensor(
                out=yt[tl % 2][:],
                in0=vt[0:FO, P : 2 * P],
                in1=vs[:],
                op=mybir.AluOpType.add,
            ).then_inc(ya_sem, 1)

        @block.scalar
        def _(scalar):
            for t in range(NTILES):
                b = t % 2
                scalar.wait_ge(dma_s[b], 48 * (t // 2 + 1))
                if t >= 2:
                    scalar.wait_ge(v_sem, t - 1)  # WAR on pr[b] scratch
                for idx, j in enumerate(range(N_DVE, D)):
                    inst = scalar.activation(
                        prb[b][:, idx * 2 * P : (idx + 1) * 2 * P],
                        src_for((xa, xb), b, j),
                        mybir.ActivationFunctionType.Copy,
                        scale=hh[b][:, j : j + 1],
                    )
                    if idx == N_ACT - 1:
                        inst.then_inc(act_sem, 1)


    _nc_cache["nc"] = nc
    return nc


def _prep_core_inputs(x, h):
    x = np.ascontiguousarray(x, dtype=np.float32)
    h = np.ascontiguousarray(h, dtype=np.float32)
    xp = np.zeros((B, TPC), np.float16)
    xp[:, PAD : PAD + T] = x.astype(np.float16)
    hpad = np.ascontiguousarray(np.concatenate([h, h[:, -1:, :]], axis=1))  # (B,N+1,D) f32
    w1 = (np.arange(P, dtype=np.float32) / P).astype(np.float16)
    w0 = (1.0 - np.arange(P, dtype=np.float32) / P).astype(np.float16)
    rr = np.broadcast_to(
        np.concatenate([w1, w0])[None, :], (128, 2 * P)
    )
    rr = np.ascontiguousarray(rr)
    in_maps = []
    for c in range(NCORES):
        sl = slice(c * S, (c + 1) * S)
        in_maps.append({"xp": xp[sl], "hc": hpad[sl], "rr": rr})
    return in_maps


def kernel(x, h, **kw):
    nc = build_nc()
    in_maps = _prep_core_inputs(x, h)
    res = run_bass_kernel_spmd(nc, in_maps, core_ids=list(range(NCORES)), **kw)
    out = np.concatenate([res.results[c]["out"] for c in range(NCORES)], axis=0)
    return np.ascontiguousarray(out, dtype=np.float32)


def kernel_traced(x, h, **kw):
    nc = build_nc()
    in_maps = _prep_core_inputs(x, h)
    res = run_bass_kernel_spmd(
        nc, in_maps, core_ids=list(range(NCORES)), trace=True, **kw
    )
    out = np.concatenate([res.results[c]["out"] for c in range(NCORES)], axis=0)
    return np.ascontiguousarray(out, dtype=np.float32), res



# revision 4
# speedup vs baseline: 1.2546x; 1.2546x over previous
"""Time-varying FIR (AllZeroDigitalFilter) on 8 TRN2 NeuronCores.

Hybrid 3-engine design:

Path 1 (PE / Tensor engine), frames: seq0[0:125) + all of seq1 (1125/core):
  Per frame g one self-loading matmul: stationary lhsT[j,i] = x[80g+i-j]
  (a [50 taps x 80 positions] Toeplitz slice of a shifted-copy SBUF
  buffer built by ONE strided DMA with partition stride -1), moving
  rhs = hT[:, g:g+2] (filters h_g, h_{g+1}) -> PSUM [80, 2] fp32:
  A_g[i], B_g[i]. Per 250-frame chunk the interpolation blend
  y = w0[i]*A + w1[i]*B runs as 2 wide DVE ops (per-partition ramp
  scalars, stride-2 PSUM APs), then PE-transposes [80,125]->[125,80]
  put y in frame-major order, ACT evacuates PSUM->SBUF fp32, and one
  contiguous DMA stores 125 frames. ldweights dominates PE time
  (~67ns/frame: load cost scales with stationary columns).

Path 2 (DVE+ACT, the prior kernel's machinery), frames seq0[125:1000):
  fp16 "C-decomposition": C_k[i'] = sum_j h[k,j] x[(k-1)P+i'-j],
  i' in [0,160); y[kP+i] = w0[i]*C_k[80+i] + w1[i]*C_{k+1}[i].
  Per 126-row tile, N_DVE taps run as scalar_tensor_tensor chains on
  Vector; the other taps are Scalar-engine products folded by a fp16
  tensor-tensor halving tree on Vector. Cross-partition combine via
  partition-shifted SBUF->SBUF DMA + one add emitting fp32.

The two paths share engines without conflict: DVE interleaves one PE
blend between successive path-2 tiles; ACT interleaves PSUM
evacuations; the SP queue interleaves both paths' DMAs so neither
path's waits block the other's issue order.

Sync design note: cumulative thresholds on a shared DMA semaphore are
unsound with >1 DMA in flight (per-SDMA-engine completion skew lets a
later tile's increments satisfy an earlier tile's threshold). Buffer-
parity semaphores make every threshold equal to the maximum possible
increment count at wait time, so a fired wait implies full completion.
"""

import sys

for p in ("/opt/trn_rl_repo", "/root/.axon_site/_ro/trn_rl_repo"):
    if p not in sys.path:
        sys.path.append(p)

import numpy as np
import concourse.bass as bass
import concourse.mybir as mybir
from concourse.ap import AP
from concourse.bass_utils import run_bass_kernel_spmd

B, T = 16, 80000
P, D = 80, 50  # frame period, taps
N = T // P  # 1000 frames
W2 = 2 * P + D - 1  # 209: extended window for the 160-wide C rows
NCORES = 8
S = B // NCORES  # sequences per core
FO = 125  # output frames per tile (path 2)
FT = FO + 1  # C-rows per tile (tiles overlap by 1 row)
PAD = D - 1 + P  # front pad so windows are in-bounds: 129
TPC = N * P + W2 + 2  # padded x length (+2 slack for the odd-offset copy)

F16 = mybir.dt.float16
FP32 = mybir.dt.float32

N_DVE = 26  # path-2 taps computed on the Vector engine

# --- PE path layout ---
# chunks: (seq, first frame, frame count); G must be a multiple of 125.
PE_CHUNKS = [(0, 0, 125)] + [(1, g0, 250) for g0 in range(0, N, 250)]
NCH = len(PE_CHUNKS)
PE_S0_FRAMES = 125  # seq0 frames handled by the PE path
NT_BASE = (N - PE_S0_FRAMES) // FO  # 7 path-2 tiles, all seq0
HTS = 1008  # ht column stride per sequence
WXS = 250 * P + P  # shifted-x chunk buffer width (max chunk)

_nc_cache = {}


def build_nc():
    if "nc" in _nc_cache:
        return _nc_cache["nc"]
    nc = bass.Bass()
    xp_ext = nc.declare_dram_parameter("xp", [S, TPC], F16, isOutput=False)
    hc_ext = nc.declare_dram_parameter("hc", [S, N + 1, D], FP32, isOutput=False)
    rr_ext = nc.declare_dram_parameter("rr", [128, 2 * P], F16, isOutput=False)
    ht_ext = nc.declare_dram_parameter("ht", [D, S * HTS], F16, isOutput=False)
    id_ext = nc.declare_dram_parameter("idt", [128, 128], F16, isOutput=False)
    wv_ext = nc.declare_dram_parameter("wv", [128, 2], FP32, isOutput=False)
    out_ext = nc.declare_dram_parameter("out", [S, T], FP32, isOutput=True)

    from contextlib import ExitStack

    with ExitStack() as _ctx:
        ec = _ctx.enter_context
        # --- path 2 (DVE+ACT) buffers ---
        xa0 = ec(nc.sbuf_tensor([FT, W2], F16))
        xa1 = ec(nc.sbuf_tensor([FT, W2], F16))
        xb0 = ec(nc.sbuf_tensor([FT, W2], F16))
        xb1 = ec(nc.sbuf_tensor([FT, W2], F16))
        hh0 = ec(nc.sbuf_tensor([FT, D], FP32))
        hh1 = ec(nc.sbuf_tensor([FT, D], FP32))
        acc0 = ec(nc.sbuf_tensor([FT, 2 * P], F16))
        acc1 = ec(nc.sbuf_tensor([FT, 2 * P], F16))
        vt = ec(nc.sbuf_tensor([FT, 2 * P], F16))
        vs = ec(nc.sbuf_tensor([FO, P], F16))
        y0 = ec(nc.sbuf_tensor([FO, P], FP32))
        y1 = ec(nc.sbuf_tensor([FO, P], FP32))
        rrt = ec(nc.sbuf_tensor([128, 2 * P], F16))
        ramp_sem = ec(nc.semaphore("ramp_sem"))
        dma_e = ec(nc.semaphore("dma_e"))
        dma_o = ec(nc.semaphore("dma_o"))
        v_sem = ec(nc.semaphore("v_sem"))
        vs_sem = ec(nc.semaphore("vs_sem"))
        ya_sem = ec(nc.semaphore("ya_sem"))
        out_e = ec(nc.semaphore("out_e"))
        out_o = ec(nc.semaphore("out_o"))
        act_sem = ec(nc.semaphore("act_sem"))
        N_ACT = D - N_DVE
        NSLOT = 32  # padded to a power of two for the in-place halving tree
        assert N_ACT <= NSLOT
        prb0 = ec(nc.sbuf_tensor([FT, NSLOT * 2 * P], F16))
        prb1 = ec(nc.sbuf_tensor([FT, NSLOT * 2 * P], F16))
        prb = [prb0, prb1]

        # --- PE path buffers ---
        xs0 = ec(nc.sbuf_tensor([D, WXS], F16))
        xs1 = ec(nc.sbuf_tensor([D, WXS], F16))
        htt = ec(nc.sbuf_tensor([D, S * HTS], F16))
        idt = ec(nc.sbuf_tensor([128, 128], F16))
        wvt = ec(nc.sbuf_tensor([128, 2], FP32))
        t1b = ec(nc.sbuf_tensor([P, 256], F16))
        yph0 = ec(nc.sbuf_tensor([P, 256], F16))
        yph1 = ec(nc.sbuf_tensor([P, 256], F16))
        yo0 = ec(nc.sbuf_tensor([FO, 2 * P], FP32))
        yo1 = ec(nc.sbuf_tensor([FO, 2 * P], FP32))
        pab0 = ec(nc.psum_tensor([P, 500], FP32))
        pab1 = ec(nc.psum_tensor([P, 500], FP32))
        pT0 = ec(nc.psum_tensor([FO, 2 * P], F16))
        pT1 = ec(nc.psum_tensor([FO, 2 * P], F16))
        const2 = ec(nc.semaphore("const2"))
        xsd0 = ec(nc.semaphore("xsd0"))
        xsd1 = ec(nc.semaphore("xsd1"))
        pe_mm = ec(nc.semaphore("pe_mm"))
        pe_tr = ec(nc.semaphore("pe_tr"))
        bl_sem = ec(nc.semaphore("bl_sem"))
        ev_sem = ec(nc.semaphore("ev_sem"))
        yst0 = ec(nc.semaphore("yst0"))
        yst1 = ec(nc.semaphore("yst1"))

        block = ec(nc.Block())
        xa = [xa0, xa1]
        xb = [xb0, xb1]
        hh = [hh0, hh1]
        yt = [y0, y1]
        dma_s = [dma_e, dma_o]
        out_s = [out_e, out_o]
        xs = [xs0, xs1]
        xsd = [xsd0, xsd1]
        pab = [pab0, pab1]
        pT = [pT0, pT1]
        yph = [yph0, yph1]
        yo = [yo0, yo1]
        yst = [yst0, yst1]

        # store-DMA bookkeeping for the PE path (buffer-parity thresholds):
        # chunk c issues G//125 store DMAs, each incrementing yst[c%2] by 16.
        def n_stores(c):
            return PE_CHUNKS[c][2] // FO

        def store_cum(c):
            # total increments on yst[c%2] after chunk c's stores complete
            return 16 * sum(n_stores(k) for k in range(c % 2, c + 1, 2))

        def xs_cum(c):
            return 16 * (c // 2 + 1)

        def ci_of(t):
            return t + 1  # path-2 tile t covers seq0 frames [(t+1)*FO, (t+2)*FO)

        def ydst(t):
            ci = ci_of(t)
            return AP(
                tensor=out_ext[:].tensor,
                offset=0 * T + ci * FO * P,
                ap=[[P, FO], [1, P]],
            )

        @block.sync
        def _(sync):
            # PE-path constants
            sync.dma_start(htt[:], ht_ext[:]).then_inc(const2, 16)
            sync.dma_start(idt[:], id_ext[:]).then_inc(const2, 16)
            sync.dma_start(wvt[:], wv_ext[:]).then_inc(const2, 16)

            def xs_dma(c):
                # partition p holds x shifted by tap j = D-1-p (stride +1;
                # negative partition steps are rejected by the verifier).
                # ht rows are flipped on the host to match this order.
                s, g0, G = PE_CHUNKS[c]
                W = G * P + P
                src = AP(
                    tensor=xp_ext[:].tensor,
                    offset=s * TPC + PAD + g0 * P - (D - 1),
                    ap=[[1, D], [1, W]],
                )
                sync.dma_start(xs[c % 2][0:D, 0:W], src).then_inc(xsd[c % 2], 16)

            def y_store(c):
                s, g0, G = PE_CHUNKS[c]
                for q in range(G // FO):
                    dst = AP(
                        tensor=out_ext[:].tensor,
                        offset=s * T + (g0 + q * FO) * P,
                        ap=[[P, FO], [1, P]],
                    )
                    sync.dma_start(dst, yo[c % 2][0:FO, q * P : (q + 1) * P]).then_inc(
                        yst[c % 2], 16
                    )

            xs_dma(0)
            xs_dma(1)

            for t in range(NT_BASE):
                ci = ci_of(t)
                b = t % 2
                k0 = ci * FO
                if t >= 2:
                    sync.wait_ge(v_sem, t - 1)  # WAR: tile t-2 read its inputs
                src_a = AP(
                    tensor=xp_ext[:].tensor,
                    offset=0 * TPC + k0 * P,
                    ap=[[P, FT], [1, W2]],
                )
                src_b = AP(
                    tensor=xp_ext[:].tensor,
                    offset=0 * TPC + k0 * P + 1,
                    ap=[[P, FT], [1, W2]],
                )
                sync.dma_start(xa[b][:], src_a).then_inc(dma_s[b], 16)
                sync.dma_start(xb[b][:], src_b).then_inc(dma_s[b], 16)
                sync.dma_start(hh[b][:], hc_ext[0, k0 : k0 + FT, :]).then_inc(
                    dma_s[b], 16
                )
                if t == 0:
                    sync.dma_start(rrt[:], rr_ext[:]).then_inc(ramp_sem, 16)
                if t >= 1:
                    # partition-shift copy of V rows 1..FT for tile t-1
                    sync.wait_ge(v_sem, t)
                    sync.dma_start(vs[:], vt[1:FT, 0:P]).then_inc(vs_sem, 16)
                if t >= 2:
                    # store y of tile t-2
                    sync.wait_ge(ya_sem, t - 1)
                    sync.dma_start(ydst(t - 2), yt[(t - 2) % 2][:]).then_inc(
                        out_s[(t - 2) % 2], 16
                    )
                # --- PE path interleaves (issued after tile t's own DMAs) ---
                if 1 <= t <= 3:
                    c = t + 1  # Xs chunks 2..4
                    sync.wait_ge(pe_mm, c - 1)  # buffer c%2 free
                    xs_dma(c)
                if 4 <= t <= 6:
                    c = t - 4  # stores for chunks 0..2
                    sync.wait_ge(ev_sem, c + 1)
                    y_store(c)

            # tail: last tile's shift + remaining stores
            tl = NT_BASE - 1
            sync.wait_ge(v_sem, NT_BASE)
            sync.dma_start(vs[:], vt[1:FT, 0:P]).then_inc(vs_sem, 16)
            sync.wait_ge(ya_sem, NT_BASE - 1)
            sync.dma_start(ydst(tl - 1), yt[(tl - 1) % 2][:]).then_inc(
                out_s[(tl - 1) % 2], 16
            )
            sync.wait_ge(ya_sem, NT_BASE)
            sync.dma_start(ydst(tl), yt[tl % 2][:]).then_inc(out_s[tl % 2], 16)
            for c in range(3, NCH):
                sync.wait_ge(ev_sem, c + 1)
                y_store(c)
            sync.wait_ge(out_s[tl % 2], 16 * (tl // 2 + 1))
            sync.wait_ge(out_s[1 - tl % 2], 16 * ((tl - 1) // 2 + 1))
            sync.wait_ge(yst[0], store_cum(4))
            sync.wait_ge(yst[1], store_cum(3))

        def src_for(buf_pair, b, j):
            # slice of the extended window for tap j, 4B-aligned via the
            # one-element-shifted copy when the natural offset is odd
            off = D - 1 - j
            if off % 2 == 0:
                return buf_pair[0][b][:, off : off + 2 * P]
            return buf_pair[1][b][:, off - 1 : off - 1 + 2 * P]

        @block.vector
        def _(vector):
            def conv(t):
                b = t % 2
                accs = [acc0, acc1]
                vector.wait_ge(dma_s[b], 48 * (t // 2 + 1))
                vector.tensor_scalar_mul(acc0[:], src_for((xa, xb), b, 0), hh[b][:, 0:1])
                cur = 0
                for j in range(1, N_DVE):
                    nxt = 1 - cur
                    vector.scalar_tensor_tensor(
                        out=accs[nxt][:],
                        in0=src_for((xa, xb), b, j),
                        scalar=hh[b][:, j : j + 1],
                        in1=accs[cur][:],
                        op0=mybir.AluOpType.mult,
                        op1=mybir.AluOpType.add,
                    )
                    cur = nxt
                # fold in the ACT-engine products
                vector.wait_ge(act_sem, t + 1)
                if N_ACT > 16:
                    extra = N_ACT - 16
                    vector.tensor_tensor(
                        out=prb[b][:, 0 : extra * 2 * P],
                        in0=prb[b][:, 0 : extra * 2 * P],
                        in1=prb[b][:, 16 * 2 * P : N_ACT * 2 * P],
                        op=mybir.AluOpType.add,
                    )
                    width = 16 * 2 * P
                else:
                    width = NSLOT * 2 * P
                while width > 2 * P:
                    half = width // 2
                    vector.tensor_tensor(
                        out=prb[b][:, 0:half],
                        in0=prb[b][:, 0:half],
                        in1=prb[b][:, half:width],
                        op=mybir.AluOpType.add,
                    )
                    width = half
                nxt = 1 - cur
                vector.tensor_tensor(
                    out=accs[nxt][:],
                    in0=accs[cur][:],
                    in1=prb[b][:, 0 : 2 * P],
                    op=mybir.AluOpType.add,
                )
                cur = nxt
                return accs[cur]

            def blend(c):
                s, g0, G = PE_CHUNKS[c]
                buf = c % 2
                vector.wait_ge(pe_mm, c + 1)
                if c >= 2:
                    vector.wait_ge(pe_tr, c - 1)  # yph[buf] WAR
                if c == 0:
                    vector.wait_ge(const2, 48)
                vector.tensor_scalar_mul(
                    t1b[0:P, 0:G], pab[buf][0:P, 1 : 2 * G : 2], wvt[0:P, 1:2]
                )
                vector.scalar_tensor_tensor(
                    out=yph[buf][0:P, 0:G],
                    in0=pab[buf][0:P, 0 : 2 * G : 2],
                    scalar=wvt[0:P, 0:1],
                    in1=t1b[0:P, 0:G],
                    op0=mybir.AluOpType.mult,
                    op1=mybir.AluOpType.add,
                ).then_inc(bl_sem, 1)

            if N_ACT <= 16:
                for pp in range(2):
                    vector.memset(prb[pp][:, N_ACT * 2 * P : NSLOT * 2 * P], 0.0)
            for t in range(NT_BASE):
                fin = conv(t)
                if t == 0:
                    vector.wait_ge(ramp_sem, 16)
                if t >= 1:
                    # combine tile t-1: y = V[0:FO, 80:160] + Vs
                    vector.wait_ge(vs_sem, 16 * t)
                    if t - 1 >= 2:
                        vector.wait_ge(out_s[(t - 1) % 2], 16 * ((t - 1) // 2))
                    vector.tensor_tensor(
                        out=yt[(t - 1) % 2][:],
                        in0=vt[0:FO, P : 2 * P],
                        in1=vs[:],
                        op=mybir.AluOpType.add,
                    ).then_inc(ya_sem, 1)
                # V_t = C_t * rr
                vector.tensor_tensor(
                    out=vt[:], in0=fin[:], in1=rrt[0:FT, :], op=mybir.AluOpType.mult
                ).then_inc(v_sem, 1)
                # --- PE-path blend interleave ---
                c = t - 1
                if 0 <= c < NCH:
                    blend(c)
            # tail combine for last tile
            tl = NT_BASE - 1
            vector.wait_ge(vs_sem, 16 * NT_BASE)
            vector.wait_ge(out_s[tl % 2], 16 * (tl // 2))
            vector.tensor_tensor(
                out=yt[tl % 2][:],
                in0=vt[0:FO, P : 2 * P],
                in1=vs[:],
                op=mybir.AluOpType.add,
            ).then_inc(ya_sem, 1)

        @block.scalar
        def _(scalar):
            def evac(c):
                s, g0, G = PE_CHUNKS[c]
                buf = c % 2
                scalar.wait_ge(pe_tr, c + 1)
                if c >= 2:
                    scalar.wait_ge(yst[buf], store_cum(c - 2))
                scalar.activation(
                    yo[buf][0:FO, 0 : (G // FO) * P],
                    pT[buf][0:FO, 0 : (G // FO) * P],
                    mybir.ActivationFunctionType.Copy,
                ).then_inc(ev_sem, 1)

            for t in range(NT_BASE):
                b = t % 2
                scalar.wait_ge(dma_s[b], 48 * (t // 2 + 1))
                if t >= 2:
                    scalar.wait_ge(v_sem, t - 1)  # WAR on prb[b] scratch
                for idx, j in enumerate(range(N_DVE, D)):
                    inst = scalar.activation(
                        prb[b][:, idx * 2 * P : (idx + 1) * 2 * P],
                        src_for((xa, xb), b, j),
                        mybir.ActivationFunctionType.Copy,
                        scale=hh[b][:, j : j + 1],
                    )
                    if idx == N_ACT - 1:
                        inst.then_inc(act_sem, 1)
                # --- PE-path evacuation interleave ---
                c = t - 2
                if 0 <= c < NCH:
                    evac(c)

        @block.tensor
        def _(tensor):
            def do_transpose(c):
                s, g0, G = PE_CHUNKS[c]
                buf = c % 2
                tensor.wait_ge(bl_sem, c + 1)  # yph ready
                if c >= 2:
                    tensor.wait_ge(ev_sem, c - 1)  # pT[buf] WAR
                nq = G // FO
                for q in range(nq):
                    tr = tensor.transpose(
                        pT[buf][0:FO, q * P : (q + 1) * P],
                        yph[buf][0:P, q * FO : (q + 1) * FO],
                        idt[0:P, 0:P],
                    )
                    if q == nq - 1:
                        tr.then_inc(pe_tr, 1)

            tensor.wait_ge(const2, 48)
            for c in range(NCH):
                s, g0, G = PE_CHUNKS[c]
                buf = c % 2
                if c >= 2:
                    tensor.wait_ge(bl_sem, c - 1)  # pab[buf] WAR
                tensor.wait_ge(xsd[buf], xs_cum(c))
                for g in range(G):
                    mm = tensor.matmul(
                        pab[buf][0:P, 2 * g : 2 * g + 2],
                        xs[buf][0:D, P * g : P * g + P],
                        htt[0:D, s * HTS + g0 + g : s * HTS + g0 + g + 2],
                        start=True,
                        stop=True,
                    )
                    if g == G - 1:
                        mm.then_inc(pe_mm, 1)
                if c >= 1:
                    do_transpose(c - 1)
            do_transpose(NCH - 1)

    _nc_cache["nc"] = nc
    return nc


def _prep_core_inputs(x, h):
    x = np.ascontiguousarray(x, dtype=np.float32)
    h = np.ascontiguousarray(h, dtype=np.float32)
    xp = np.zeros((B, TPC), np.float16)
    xp[:, PAD : PAD + T] = x.astype(np.float16)
    hpad = np.ascontiguousarray(np.concatenate([h, h[:, -1:, :]], axis=1))  # (B,N+1,D) f32
    w1 = (np.arange(P, dtype=np.float32) / P).astype(np.float16)
    w0 = (1.0 - np.arange(P, dtype=np.float32) / P).astype(np.float16)
    rr = np.broadcast_to(np.concatenate([w1, w0])[None, :], (128, 2 * P))
    rr = np.ascontiguousarray(rr)
    hpad16 = hpad.astype(np.float16)  # (B, N+1, D)
    idt = np.eye(128, dtype=np.float16)
    wv = np.zeros((128, 2), np.float32)
    wv[0:P, 0] = 1.0 - np.arange(P, dtype=np.float32) / P
    wv[0:P, 1] = np.arange(P, dtype=np.float32) / P
    in_maps = []
    for c in range(NCORES):
        sl = slice(c * S, (c + 1) * S)
        ht = np.zeros((D, S * HTS), np.float16)
        for s in range(S):
            # row p = tap D-1-p, matching the stride +1 shifted-x layout
            ht[:, s * HTS : s * HTS + N + 1] = hpad16[c * S + s].T[::-1, :]
        in_maps.append(
            {
                "xp": xp[sl],
                "hc": hpad[sl],
                "rr": rr,
                "ht": ht,
                "idt": idt,
                "wv": wv,
            }
        )
    return in_maps


def kernel(x, h, **kw):
    nc = build_nc()
    in_maps = _prep_core_inputs(x, h)
    res = run_bass_kernel_spmd(nc, in_maps, core_ids=list(range(NCORES)), **kw)
    out = np.concatenate([res.results[c]["out"] for c in range(NCORES)], axis=0)
    return np.ascontiguousarray(out, dtype=np.float32)


def kernel_traced(x, h, **kw):
    nc = build_nc()
    in_maps = _prep_core_inputs(x, h)
    res = run_bass_kernel_spmd(
        nc, in_maps, core_ids=list(range(NCORES)), trace=True, **kw
    )
    out = np.concatenate([res.results[c]["out"] for c in range(NCORES)], axis=0)
    return np.ascontiguousarray(out, dtype=np.float32), res
